# revision 1
# baseline (speedup 1.0000x reference)
"""Distributed windowed-attention kernel for 8 NeuronCores.

Sharding (tensor-parallel over heads, per the hint): B=2, nh=12 -> 24
(batch, head) attention units, 3 per core. Core c handles batch b = c//4
and heads 3*(c%4) .. 3*(c%4)+2. Each core computes q/k/v projections for
its heads, decomposed rel-pos attention, and its partial output
projection [N, C]. The host gather sums the 4 partials per batch
(the TP reduce) and adds proj_b.

Everything runs on the 8 NeuronCores via the JAX/axon PJRT backend as a
single SPMD pmap program.
"""

import numpy as np
import jax
import jax.numpy as jnp
from functools import partial

B, H_DIM, W_DIM, C, HEAD_DIM = 2, 48, 48, 768, 64
N = H_DIM * W_DIM          # 2304
NH = C // HEAD_DIM         # 12
N_CORES = 8
CORES_PER_B = N_CORES // B  # 4
HEADS_PER_CORE = NH // CORES_PER_B  # 3
SCALE = HEAD_DIM ** -0.5

_COMPILED = {}


@partial(jax.pmap, axis_name="cores")
def _attn_shard(x, wq, bq, wk, bk, wv, bv, Rh, Rw, pw):
    # x [N, C]; wq/wk/wv [h, hd, C]; bq/bk/bv [h, hd]
    # Rh [H, H, hd]; Rw [W, W, hd]; pw [h, hd, C]
    q = jnp.einsum("nc,hdc->hnd", x, wq) + bq[:, None, :]
    k = jnp.einsum("nc,hdc->hnd", x, wk) + bk[:, None, :]
    v = jnp.einsum("nc,hdc->hnd", x, wv) + bv[:, None, :]

    attn = jnp.einsum("hqd,hkd->hqk", q * SCALE, k)          # [h, N, N]

    rq = q.reshape(HEADS_PER_CORE, H_DIM, W_DIM, HEAD_DIM)
    rel_h = jnp.einsum("hxwc,xkc->hxwk", rq, Rh)             # [h, H, W, H]
    rel_w = jnp.einsum("hxwc,wkc->hxwk", rq, Rw)             # [h, H, W, W]
    attn = (attn.reshape(HEADS_PER_CORE, H_DIM, W_DIM, H_DIM, W_DIM)
            + rel_h[..., None]
            + rel_w[:, :, :, None, :]).reshape(HEADS_PER_CORE, N, N)

    attn = jax.nn.softmax(attn, axis=-1)
    o = jnp.einsum("hqk,hkd->hqd", attn, v)                  # [h, N, hd]
    return jnp.einsum("hnd,hdc->nc", o, pw)                  # partial [N, C]


def kernel(x, qkv_w, qkv_b, proj_w, proj_b, rel_pos_h, rel_pos_w, H, W):
    x = np.asarray(x, dtype=np.float32)
    qkv_w = np.asarray(qkv_w, dtype=np.float32)
    qkv_b = np.asarray(qkv_b, dtype=np.float32)
    proj_w = np.asarray(proj_w, dtype=np.float32)
    proj_b = np.asarray(proj_b, dtype=np.float32)

    # Host-side shard prep (cheap): slice weights per head, gather the
    # relative-position tables once.
    wq_full = qkv_w[0 * C:1 * C].reshape(NH, HEAD_DIM, C)
    wk_full = qkv_w[1 * C:2 * C].reshape(NH, HEAD_DIM, C)
    wv_full = qkv_w[2 * C:3 * C].reshape(NH, HEAD_DIM, C)
    bq_full = qkv_b[0 * C:1 * C].reshape(NH, HEAD_DIM)
    bk_full = qkv_b[1 * C:2 * C].reshape(NH, HEAD_DIM)
    bv_full = qkv_b[2 * C:3 * C].reshape(NH, HEAD_DIM)
    # proj rows per head: out_h [N, hd] @ pw[h] -> [N, C]
    pw_full = proj_w.T.reshape(NH, HEAD_DIM, C)

    ch = np.arange(H_DIM)[:, None] - np.arange(H_DIM)[None, :] + (H_DIM - 1)
    cw = np.arange(W_DIM)[:, None] - np.arange(W_DIM)[None, :] + (W_DIM - 1)
    Rh = np.asarray(rel_pos_h, dtype=np.float32)[ch]         # [H, H, hd]
    Rw = np.asarray(rel_pos_w, dtype=np.float32)[cw]         # [W, W, hd]

    xs, wqs, bqs, wks, bks, wvs, bvs, Rhs, Rws, pws = ([] for _ in range(10))
    for c in range(N_CORES):
        b = c // CORES_PER_B
        h0 = (c % CORES_PER_B) * HEADS_PER_CORE
        sl = slice(h0, h0 + HEADS_PER_CORE)
        xs.append(x[b])
        wqs.append(wq_full[sl]); bqs.append(bq_full[sl])
        wks.append(wk_full[sl]); bks.append(bk_full[sl])
        wvs.append(wv_full[sl]); bvs.append(bv_full[sl])
        Rhs.append(Rh); Rws.append(Rw)
        pws.append(pw_full[sl])

    stack = lambda lst: np.stack(lst, axis=0)
    partials = _attn_shard(stack(xs), stack(wqs), stack(bqs), stack(wks),
                           stack(bks), stack(wvs), stack(bvs), stack(Rhs),
                           stack(Rws), stack(pws))
    partials = np.asarray(partials)                          # [8, N, C]

    # Host gather/unshard: TP reduce of the 4 per-batch partials + bias.
    out = np.empty((B, N, C), dtype=np.float32)
    for b in range(B):
        out[b] = partials[b * CORES_PER_B:(b + 1) * CORES_PER_B].sum(axis=0)
        out[b] += proj_b[None, :]
    return out

